# revision 21
# baseline (speedup 1.0000x reference)
"""Trainium2 Bass kernel for CausalSelfAttention (B=2, T=2048, C=1024, H=16).

Sharding: 8 cores = 2 batches x 4 head-groups (4 heads each).

Per core the two head-pairs' attention blocks are interleaved round-robin
so the Act-engine exp chain of one pair overlaps the PE matmuls of the
other, and qkv(n+1)/proj(n-2) matmuls are drained into the attention
rounds as PE filler (keeps the PE HAM-warm).  Act runs (almost) only exp;
PSUM->SBUF copies are split between Act and DVE; softmax denominators are
reduced on the PE (0.9-weighted ones matmul) and reciprocals read PSUM
directly.  All DRAM operands are host-prepacked into the exact SBUF
layout so every load is one contiguous DMA; the dropout mask streams in
2-block chunks.  y is shipped once per token-slice through a single
4-core AllGather (both pairs together).
"""

import sys

sys.path.insert(0, "/opt/trn_rl_repo")

from collections import deque

import numpy as np

import concourse.bass as bass
import concourse.mybir as mybir
import concourse.tile as tile
from concourse import bacc
from concourse.bass_utils import run_bass_kernel_spmd

F32 = mybir.dt.float32
F16 = mybir.dt.float16

B, T, C, H = 2, 2048, 1024, 16
HD = C // H  # 64
N_CORES = 8
GROUPS = 4            # head groups (one per core within a batch)
HPG = H // GROUPS     # heads per group = 4
PAIRS = HPG // 2      # head pairs per core = 2
KEEP = 0.9
EXP_BIAS = -3.0       # exp(s - 3): cancels in normalization, avoids overflow

NT = T // 512         # 4 token slices of 512
KT = C // 128         # 8 contraction tiles
XR = 3                # x slice ring depth
YR = 2                # yall slice ring depth

# mask_lin column offset of slice j (per pair region is 40960 cols)
MOFF = [0, 4 * 1024, 12 * 1024, 24 * 1024]
MPAIR = 40 * 1024

DEBUG = False  # dump csum/den/yj per slice for debugging


def build_kernel():
    nc = bacc.Bacc("TRN2", target_bir_lowering=False, debug=False,
                   num_devices=N_CORES)

    # ---- per-core DRAM I/O (all host-prepacked to SBUF layout) ----
    xlin = nc.dram_tensor("xlin", [128, KT * T], F16, kind="ExternalInput")
    wqk = nc.dram_tensor("wqk", [128, KT * 512], F16, kind="ExternalInput")
    wv = nc.dram_tensor("wv", [128, KT * 256], F16, kind="ExternalInput")
    wproj = nc.dram_tensor("wproj", [128, KT * 256], F16, kind="ExternalInput")
    vbias = nc.dram_tensor("vbias", [128, 256], F32, kind="ExternalInput")
    bqk = nc.dram_tensor("bqk", [128, 4], F32, kind="ExternalInput")
    bpr = nc.dram_tensor("bpr", [128, 2], F32, kind="ExternalInput")
    rmask = nc.dram_tensor("rmask", [128, 2 * T], F16, kind="ExternalInput")
    stair = nc.dram_tensor("stair", [128, 4 * 1024], F16, kind="ExternalInput")
    mask_lin = nc.dram_tensor("mask_lin", [128, 2 * MPAIR], F16,
                              kind="ExternalInput")
    out = nc.dram_tensor("out", [256, T], F32, kind="ExternalOutput")
    dbg = None
    if DEBUG:
        dbg = dict(
            csum=nc.dram_tensor("dbg_csum", [128, NT * 2048], F16,
                                kind="ExternalOutput"),
            den=nc.dram_tensor("dbg_den", [64, NT * 2048], F32,
                               kind="ExternalOutput"),
            yj=nc.dram_tensor("dbg_yj", [128, NT * 1024], F16,
                              kind="ExternalOutput"),
        )

    # internal DRAM for the per-slice AllGather (both pairs together)
    ag_in = [nc.dram_tensor(f"ag_in{j}", [256, 512], F16) for j in range(NT)]
    ag_out = [nc.dram_tensor(f"ag_out{j}", [1024, 512], F16)
              for j in range(NT)]

    with tile.TileContext(nc) as tc:
        _build_body(nc, tc, xlin, wqk, wv, wproj, vbias, bqk, bpr, rmask,
                    stair, mask_lin, out, ag_in, ag_out, dbg)
    nc.compile()
    return nc


def _build_body(nc, tc, xlin, wqk, wv, wproj, vbias, bqk, bpr, rmask,
                stair, mask_lin, out, ag_in, ag_out, dbg=None):
    from contextlib import ExitStack
    ctx = ExitStack()
    AF = mybir.ActivationFunctionType
    ALU = mybir.AluOpType

    # ---- PSUM (8 banks) ----
    ps_s = [ctx.enter_context(nc.psum_tensor(f"ps_s{p}", [128, 1024], F32))
            for p in range(PAIRS)]                               # S per pair
    ps_av = [ctx.enter_context(nc.psum_tensor(f"ps_av{h}", [128, 512], F32))
             for h in range(2)]                                  # AV per head
    ps_u = ctx.enter_context(nc.psum_tensor([128, 512], F32))    # qkv/proj
    ps_w = ctx.enter_context(nc.psum_tensor([128, 512], F32))

    # ---- persistent SBUF ----
    big = ctx.enter_context(tc.tile_pool(name="big", bufs=1))
    xT_sb = big.tile([128, KT * 512 * XR], F16, name="xT")   # x ring, k-major
    yall_sb = big.tile([128, KT * 512 * YR], F16, name="ya")  # y ring, k-major
    qT_sb = [big.tile([128, T], F16, name=f"qT{p}") for p in range(PAIRS)]
    kT_sb = [big.tile([128, T], F16, name=f"kT{p}") for p in range(PAIRS)]
    v_sb = big.tile([128, (T // 128) * 256], F16, name="v")
    wqk_sb = big.tile([128, KT * 512], F16, name="wqk")
    wv_sb = big.tile([128, KT * 256], F16, name="wv")
    wproj_sb = big.tile([128, KT * 256], F16, name="wp")
    vbias_sb = big.tile([128, 256], F32, name="vbias")
    bqk_sb = big.tile([128, 4], F32, name="bqk")
    bpr_sb = big.tile([128, 2], F32, name="bpr")
    rmask_sb = big.tile([128, 2 * T], F16, name="rm")
    stair_sb = big.tile([128, 4 * 1024], F16, name="stair")
    ones09 = big.tile([128, 128], F16, name="ones09")
    expb_sb = big.tile([128, 1], F32, name="expb")

    # ---- rotating SBUF pools ----
    mpool = [ctx.enter_context(tc.tile_pool(name=f"mask{p}", bufs=3))
             for p in range(PAIRS)]
    apool = ctx.enter_context(tc.tile_pool(name="araw", bufs=6))
    cpool = ctx.enter_context(tc.tile_pool(name="csum", bufs=4))
    rpool = ctx.enter_context(tc.tile_pool(name="recip", bufs=2))
    ypool = ctx.enter_context(tc.tile_pool(name="yj", bufs=4))
    opool = ctx.enter_context(tc.tile_pool(name="oproj", bufs=2))

    def xcol(n, k):
        return k * (512 * XR) + (n % XR) * 512

    def ycol(j, k):
        return k * (512 * YR) + (j % YR) * 512

    def load_x_slice(n):
        dst = xT_sb[:].rearrange("p (k q) -> p k q", k=KT)[:, :,
                                                           (n % XR) * 512:
                                                           (n % XR + 1) * 512]
        src = xlin.ap().rearrange("p (k t) -> p k t", k=KT)[:, :,
                                                            n * 512:
                                                            (n + 1) * 512]
        nc.sync.dma_start(dst, src)

    # ---- preamble loads ----
    nc.sync.dma_start(wqk_sb[:], wqk[:, :])
    load_x_slice(0)
    nc.sync.dma_start(wv_sb[:], wv[:, :])
    nc.sync.dma_start(vbias_sb[:], vbias[:, :])
    nc.sync.dma_start(bqk_sb[:], bqk[:, :])
    load_x_slice(1)
    nc.sync.dma_start(stair_sb[:], stair[:, :])
    nc.sync.dma_start(bpr_sb[:], bpr[:, :])
    nc.sync.dma_start(rmask_sb[:], rmask[:, :])
    nc.sync.dma_start(wproj_sb[:], wproj[:, :])
    nc.vector.memset(ones09[:], KEEP)
    nc.vector.memset(expb_sb[:], EXP_BIAS)

    # alternate PSUM bank and copy engine for qkv/proj fills
    state = {"pp": 0, "ce": 0}

    def next_ps():
        state["pp"] ^= 1
        return ps_u if state["pp"] else ps_w

    def copy_engine():
        state["ce"] ^= 1
        return state["ce"]

    def qkv_items(n):
        """Work items (closures) for the QKV projection of slice n."""
        items = []
        # Q^T/K^T: m 0=q-pair0, 1=q-pair1, 2=k-pair0, 3=k-pair1
        for m in range(4):
            ps = next_ps()

            def mk(m=m, ps=ps, lo=0):
                def run():
                    for k in range(4 * lo, 4 * lo + 4):
                        nc.tensor.matmul(
                            ps[:],
                            wqk_sb[:, k * 512 + m * 128:
                                   k * 512 + (m + 1) * 128],
                            xT_sb[:, xcol(n, k):xcol(n, k) + 512],
                            start=(k == 0), stop=(k == KT - 1))
                    if lo == 1:
                        dest = (qT_sb if m < 2 else kT_sb)[m % 2]
                        dsl = dest[:, n * 512:(n + 1) * 512]
                        if copy_engine():
                            nc.scalar.add(dsl, ps[:], bqk_sb[:, m:m + 1])
                        else:
                            nc.vector.tensor_scalar(dsl, ps[:],
                                                    bqk_sb[:, m:m + 1],
                                                    None, ALU.add)
                return run
            items.append(mk(lo=0))
            items.append(mk(lo=1))
        # V: natural layout [tok, vfeat]
        for t in range(4):
            q = 4 * n + t
            ps = next_ps()

            def mkv(q=q, ps=ps, lo=0):
                def run():
                    for k in range(4 * lo, 4 * lo + 4):
                        nc.tensor.matmul(
                            ps[:, 0:256],
                            xT_sb[:, xcol(n, k) + (q % 4) * 128:
                                  xcol(n, k) + (q % 4) * 128 + 128],
                            wv_sb[:, k * 256:(k + 1) * 256],
                            start=(k == 0), stop=(k == KT - 1))
                    if lo == 1:
                        nc.vector.tensor_tensor(
                            v_sb[:, q * 256:(q + 1) * 256], ps[:, 0:256],
                            vbias_sb[:], ALU.add)
                return run
            items.append(mkv(lo=0))
            items.append(mkv(lo=1))
        return items

    def gather_reads(j):
        """Pull the AllGathered y slice into the yall ring (gpsimd queue;
        data dependency on ag_out[j] orders it after the collective)."""
        dst = yall_sb[:].rearrange("p (k q) -> p k q", k=KT)[
            :, :, (j % YR) * 512:(j % YR + 1) * 512]
        src = ag_out[j].ap().rearrange("(k p) q -> p k q", k=KT)
        nc.gpsimd.dma_start(dst, src)

    def proj_items(j):
        """Work items for the output projection of token slice j."""
        items = []
        for m in range(2):
            ps = next_ps()

            def mk(m=m, ps=ps, lo=0):
                def run():
                    for k in range(4 * lo, 4 * lo + 4):
                        nc.tensor.matmul(
                            ps[:],
                            wproj_sb[:, k * 256 + m * 128:
                                     k * 256 + (m + 1) * 128],
                            yall_sb[:, ycol(j, k):ycol(j, k) + 512],
                            start=(k == 0), stop=(k == KT - 1))
                    if lo == 1:
                        o_m = opool.tile([128, 512], F32, tag="oproj")
                        nc.vector.scalar_tensor_tensor(
                            o_m[:], ps[:], bpr_sb[:, m:m + 1],
                            rmask_sb[:, m * T + j * 512:
                                     m * T + (j + 1) * 512],
                            ALU.add, ALU.mult)
                        nc.sync.dma_start(
                            out[m * 128:(m + 1) * 128,
                                j * 512:(j + 1) * 512], o_m[:])
                return run
            items.append(mk(lo=0))
            items.append(mk(lo=1))
        return items

    def attn(j, work):
        """Causal attention for query slice j, both pairs interleaved
        per tk-block round; `work` items are drained as PE filler."""
        n_i = 4 * (j + 1)
        nch = n_i // 2  # 2-block mask chunks per pair
        csum = [cpool.tile([128, 1024], F16, tag=f"cs{p}", name=f"cs{p}")
                for p in range(PAIRS)]
        chunks = [[None] * nch for _ in range(PAIRS)]

        def load_chunk(p, c):
            mt = mpool[p].tile([128, 2048], F16, tag=f"mc{p}")
            off = p * MPAIR + MOFF[j] + c * 2048
            nc.sync.dma_start(mt[:], mask_lin[:, off:off + 2048])
            chunks[p][c] = mt

        for p in range(PAIRS):
            for c in range(min(2, nch)):
                load_chunk(p, c)

        per_round = max(1, -(-len(work) // n_i))  # ceil
        pend = [deque() for _ in range(PAIRS)]

        def av_block(p, i, a):
            # the start=True psum clear is per-partition (HW-probed), so
            # the two pairs' groups coexist in each bank's row halves
            for h in range(2):
                nc.tensor.matmul(
                    ps_av[h][64 * p:64 * p + 64, :],
                    v_sb[:, i * 256 + (2 * p + h) * 64:
                         i * 256 + (2 * p + h) * 64 + 64],
                    a[:, h * 512:(h + 1) * 512],
                    start=(i == 0), stop=(i == n_i - 1),
                    skip_group_check=True)

        for i in range(n_i):
            for p in range(PAIRS):
                # S^T for block i: out [tk=128, tq 2x512], heads in the
                # two PE row-groups concurrently
                for h in range(2):
                    nc.tensor.matmul(
                        ps_s[p][:, h * 512:(h + 1) * 512],
                        kT_sb[p][h * 64:(h + 1) * 64,
                                 i * 128:(i + 1) * 128],
                        qT_sb[p][h * 64:(h + 1) * 64,
                                 j * 512:(j + 1) * 512],
                        start=True, stop=True)
                a = apool.tile([128, 1024], F16, tag="araw")
                nc.scalar.activation(a[:], ps_s[p][:], AF.Exp,
                                     bias=expb_sb[:, 0:1], scale=1.0)
                if i >= 4 * j:  # diagonal: zero the non-causal region
                    r = i - 4 * j
                    nc.vector.tensor_tensor(
                        a[:], a[:],
                        stair_sb[:, r * 1024:(r + 1) * 1024], ALU.mult)
                if i == 0:
                    nc.vector.tensor_scalar(csum[p][:], a[:], 1.0, None,
                                            ALU.mult)
                else:
                    nc.vector.tensor_tensor(csum[p][:], csum[p][:], a[:],
                                            ALU.add)
                # dropout mask multiply (in place; mask includes tril)
                mt = chunks[p][i // 2]
                nc.vector.tensor_tensor(
                    a[:], a[:], mt[:, (i % 2) * 1024:(i % 2 + 1) * 1024],
                    ALU.mult)
                pend[p].append((i, a))
                if len(pend[p]) > 2:
                    av_block(p, *pend[p].popleft())
            for _ in range(per_round):
                if work:
                    work.popleft()()
            if i % 2 == 1:
                c_next = i // 2 + 2
                if c_next < nch:
                    for p in range(PAIRS):
                        load_chunk(p, c_next)
        while work:
            work.popleft()()
        for p in range(PAIRS):
            while pend[p]:
                av_block(p, *pend[p].popleft())
        # denominator 0.9*sum on PE at partitions 0:64.  HW quirks (probed):
        # reciprocal_approx_fast only works from SBUF at base partition 0,
        # so both pairs' den/recip run at rows 0:64 and pair 1's reciprocal
        # is then partition-shifted to rows 64:128 by an SBUF->SBUF DMA to
        # line up with its ps_av rows.
        den = rpool.tile([64, 2048], F32, tag="den")
        rec = rpool.tile([64, 2048], F32, tag="rec")
        rec_hi = rpool.tile([128, 1024], F32, tag="rec_hi")
        yj = ypool.tile([128, 1024], F16, tag="yj")
        for p in range(PAIRS):
            for h in range(2):
                nc.tensor.matmul(
                    ps_s[p][0:64, h * 512:(h + 1) * 512],
                    ones09[:, 0:64],
                    csum[p][:, h * 512:(h + 1) * 512],
                    start=True, stop=True, skip_group_check=True)
            csl = slice(p * 1024, (p + 1) * 1024)
            if p == 0:
                nc.scalar.copy(den[:, csl], ps_s[p][0:64, :])
            else:
                nc.vector.tensor_scalar(den[:, csl], ps_s[p][0:64, :],
                                        1.0, None, ALU.mult)
            nc.vector.reciprocal_approx_fast(rec[:, csl], den[:, csl])
        nc.sync.dma_start(rec_hi[64:128, :], rec[:, 1024:2048])
        for p in range(PAIRS):
            for h in range(2):
                sl = (slice(64 * p, 64 * p + 64),
                      slice(h * 512, (h + 1) * 512))
                r_ap = rec[0:64, h * 512:(h + 1) * 512] if p == 0 \
                    else rec_hi[64:128, h * 512:(h + 1) * 512]
                nc.vector.tensor_tensor(
                    yj[sl], ps_av[h][64 * p:64 * p + 64, :], r_ap,
                    ALU.mult)
            # ship this pair's 128 feature rows: ag row p*128 + h*64 + f
            for h in range(2):
                nc.sync.dma_start(
                    ag_in[j].ap()[p * 128 + h * 64:p * 128 + h * 64 + 64, :],
                    yj[64 * p:64 * p + 64, h * 512:(h + 1) * 512])
        if dbg is not None:
            for p in range(PAIRS):
                nc.sync.dma_start(
                    dbg["csum"][:, j * 2048 + p * 1024:
                                j * 2048 + (p + 1) * 1024], csum[p][:])
            nc.sync.dma_start(dbg["den"][:, j * 2048:(j + 1) * 2048], den[:])
            nc.sync.dma_start(dbg["yj"][:, j * 1024:(j + 1) * 1024], yj[:])
        nc.gpsimd.collective_compute(
            "AllGather", mybir.AluOpType.bypass,
            replica_groups=[[0, 1, 2, 3], [4, 5, 6, 7]],
            ins=[ag_in[j].ap()], outs=[ag_out[j].ap()])

    # ---- main schedule ----
    for it in qkv_items(0):
        it()
    load_x_slice(2)
    attn(0, deque(qkv_items(1)))
    load_x_slice(3)
    gather_reads(0)
    attn(1, deque(qkv_items(2)))
    gather_reads(1)
    attn(2, deque(qkv_items(3) + proj_items(0)))
    gather_reads(2)
    attn(3, deque(proj_items(1) + proj_items(2)))
    gather_reads(3)
    for it in proj_items(3):
        it()

    ctx.close()


def _pack_k(a):
    """[C, N] -> [128, KT*N] with k-major columns."""
    n = a.shape[1]
    return np.ascontiguousarray(
        a.reshape(KT, 128, n).transpose(1, 0, 2).reshape(128, KT * n))


def prep_inputs(x, Wqkv, bqkv, Wproj, bproj, attn_drop_mask, resid_drop_mask):
    """Shard + prepack the full inputs for the 8 cores."""
    x = np.asarray(x, np.float32)
    Wqkv = np.asarray(Wqkv, np.float32)
    bqkv = np.asarray(bqkv, np.float32)
    Wproj = np.asarray(Wproj, np.float32)
    bproj = np.asarray(bproj, np.float32)
    attn_drop_mask = np.asarray(attn_drop_mask, bool)
    resid_drop_mask = np.asarray(resid_drop_mask, bool)

    tril = np.tril(np.ones((T, T), dtype=bool))
    qscale = np.float32(1.0 / np.sqrt(HD))

    # stair_r[p, h*512+c] = 1 if c >= 128*r + p
    pp = np.arange(128)[:, None]
    cc = np.arange(512)[None, :]
    stair = np.zeros((128, 4 * 1024), np.float16)
    for r in range(4):
        s = (cc >= 128 * r + pp).astype(np.float16)
        stair[:, r * 1024:r * 1024 + 512] = s
        stair[:, r * 1024 + 512:(r + 1) * 1024] = s

    in_maps = []
    for core in range(N_CORES):
        b, g = divmod(core, GROUPS)
        cs = slice(g * 256, (g + 1) * 256)
        wq_c = Wqkv[:, 0:1024][:, cs] * qscale
        wk_c = Wqkv[:, 1024:2048][:, cs]
        wqk_c = np.concatenate([wq_c, wk_c], axis=1).astype(np.float16)
        wv_c = Wqkv[:, 2048:3072][:, cs].astype(np.float16)
        bq = (bqkv[0:1024][cs] * qscale).astype(np.float32)
        bk = bqkv[1024:2048][cs].astype(np.float32)
        bv = bqkv[2048:3072][cs].astype(np.float32)
        bqk_c = np.stack([bq[0:128], bq[128:256],
                          bk[0:128], bk[128:256]], axis=1)
        vbias_c = np.broadcast_to(bv, (128, 256)).astype(np.float32).copy()
        # combined causal & dropout mask, [head, tk, tq]
        m = attn_drop_mask[b, g * HPG:(g + 1) * HPG] & tril
        mt = m.transpose(0, 2, 1)
        mask_l = np.zeros((128, 2 * MPAIR), np.float16)
        for p in range(PAIRS):
            for j in range(NT):
                n_i = 4 * (j + 1)
                sub = mt[2 * p:2 * p + 2, 0:n_i * 128,
                         j * 512:(j + 1) * 512]
                sub = sub.reshape(2, n_i, 128, 512).transpose(2, 1, 0, 3)
                off = p * MPAIR + MOFF[j]
                mask_l[:, off:off + n_i * 1024] = \
                    sub.reshape(128, n_i * 1024).astype(np.float16)
        rmask_c = np.ascontiguousarray(
            resid_drop_mask[b, :, cs].T).astype(np.float16)  # [256, T]
        rmask_l = rmask_c.reshape(2, 128, T).transpose(1, 0, 2)
        rmask_l = np.ascontiguousarray(rmask_l.reshape(128, 2 * T))
        wproj_c = (Wproj[:, cs] / np.float32(KEEP)).astype(np.float16)
        bpr_c = np.stack([bproj[cs][0:128], bproj[cs][128:256]],
                         axis=1).astype(np.float32) / np.float32(KEEP)
        xT_c = np.ascontiguousarray(x[b].T).astype(np.float16)  # [C, T]
        in_maps.append(dict(
            xlin=_pack_k(xT_c),
            wqk=_pack_k(wqk_c),
            wv=_pack_k(wv_c),
            wproj=_pack_k(wproj_c),
            vbias=vbias_c,
            bqk=np.ascontiguousarray(bqk_c),
            bpr=np.ascontiguousarray(bpr_c),
            rmask=rmask_l,
            stair=stair,
            mask_lin=mask_l,
        ))
    return in_maps


_NC_CACHE = {}


def _get_nc():
    if "nc" not in _NC_CACHE:
        _NC_CACHE["nc"] = build_kernel()
    return _NC_CACHE["nc"]


def kernel(trace=False, **inputs):
    nc = _get_nc()
    in_maps = prep_inputs(**inputs)
    res = run_bass_kernel_spmd(nc, in_maps, core_ids=list(range(N_CORES)),
                               trace=trace)
    y = np.empty((B, T, C), np.float32)
    for core in range(N_CORES):
        b, g = divmod(core, GROUPS)
        y[b, :, g * 256:(g + 1) * 256] = res.results[core]["out"].T
    kernel.last_result = res
    return y
